# revision 1
# baseline (speedup 1.0000x reference)
"""Fused single-head CNN self-attention kernel for Trainium2 (8 NeuronCores).

Computes, per batch b:
    q = (Wq/sqrt(C)) @ x + bq/sqrt(C)   (Cqk=32, N=4096)
    k = Wk @ x + bk
    v = Wv @ x + bv
    E[i, j]  = q[:, i] . k[:, j]        (already scaled by 1/sqrt(C))
    P        = softmax_j(E)
    out[c,i] = gamma * sum_j P[i, j] v[c, j] + x[c, i]

Sharding: B=4 batches x 2 query-halves -> 8 cores, no cross-core comms.
Each core handles one batch's full keys/values and 2048 queries.

Device-side layout tricks:
  * Wq^T/Wk^T are replicated 4x along columns on the host so the Q/K
    projections produce Q_rep/K_rep with 4 partition-group copies; the
    energy matmul (contraction depth 32) then packs 4 concurrent
    matmuls into the 128x128 PE array via tile_position row tiling.
  * Energy is computed transposed, E^T[key, query], so exp(E^T) is
    already the stationary operand layout the P@V matmul needs; no
    transposes anywhere in the kernel.
  * V is projected directly transposed (V^T[n, c]) with an extra ones
    column, so the P@V matmul's PSUM output column 256 accumulates the
    softmax denominator for free.
  * Softmax skips max-subtraction: E = q.k/sqrt(C) with unit-variance
    inputs is bounded (|E| < ~3), far from fp32 exp overflow.
"""

import os

import numpy as np
import ml_dtypes

import concourse.bass as bass
import concourse.mybir as mybir
from concourse import bacc
from concourse.tile import TileContext
from concourse.bass_utils import run_bass_kernel_spmd

# Problem shape (hardcoded per contest contract).
B, C, H, W = 4, 256, 64, 64
N = H * W          # 4096 keys per batch
D = 32             # q/k head dim
NCORES = 8
MQ = N // 2        # 2048 queries per core
MQ_CHUNK = 512     # query strip width (PSUM bank = 512 fp32)
NBLK = N // 128    # 32 key blocks
NSTRIP = MQ // MQ_CHUNK  # 4

F32 = mybir.dt.float32
BF16 = mybir.dt.bfloat16
WARMUP_MMS = int(os.environ.get("KERNEL_WARMUP_MMS", "7"))

# Module-level stash of the last run's results (exec_time_ns etc.) so a
# test harness can report HW time without changing kernel()'s signature.
last_results = None
_nc_cache = {}


def _build_nc(has_bq, has_bk, has_bv):
    nc = bacc.Bacc(None)

    # xb is the core's batch with its 2048 query columns rotated to the
    # front (softmax over keys is permutation-invariant), so the query
    # slice is the compile-time-constant columns 0:MQ of xb.
    xb_d = nc.declare_dram_parameter("xb", [C, N], BF16, isOutput=False)
    xqt_d = nc.declare_dram_parameter("xqt", [MQ, C], F32, isOutput=False)
    wqt_d = nc.declare_dram_parameter("wqt4", [C, 128], BF16, isOutput=False)
    wkt_d = nc.declare_dram_parameter("wkt4", [C, 128], BF16, isOutput=False)
    wvt_d = nc.declare_dram_parameter("wvt", [C, C], BF16, isOutput=False)
    if has_bq:
        bq_d = nc.declare_dram_parameter("bq4", [128, 1], F32, isOutput=False)
    if has_bk:
        bk_d = nc.declare_dram_parameter("bk4", [128, 1], F32, isOutput=False)
    if has_bv:
        bv_d = nc.declare_dram_parameter("bvg", [128, C], F32, isOutput=False)
    out_d = nc.declare_dram_parameter("out", [MQ, C], F32, isOutput=True)

    with TileContext(nc) as tc:
        with (
            tc.tile_pool(name="const", bufs=1) as const,
            tc.tile_pool(name="acts", bufs=1) as acts,
            tc.tile_pool(name="ptp", bufs=26) as ptp,
            tc.tile_pool(name="outp", bufs=4) as outp,
        ):
            # ---- load weights + activations --------------------------------
            # One DMA trigger per tensor (triggers cost ~600ns each on SyncE
            # and serialize); order so Q-projection inputs land first.
            wq_sb = const.tile([128, 2, 128], BF16)
            wk_sb = const.tile([128, 2, 128], BF16)
            wv_sb = const.tile([128, 2, C], BF16)
            xb_sb = acts.tile([128, 2, N], BF16)
            # Spread loads across the two HWDGE queues (sync + scalar);
            # xb eighths land in consumption order (queries = quarters 0-1).
            nc.sync.dma_start(out=wq_sb, in_=wqt_d[:].rearrange("(t p) m -> p t m", p=128))
            nc.scalar.dma_start(out=wk_sb, in_=wkt_d[:].rearrange("(t p) m -> p t m", p=128))
            for nh in range(8):
                for t in range(2):
                    eng = nc.sync if t == 0 else nc.scalar
                    eng.dma_start(
                        out=xb_sb[:, t, 512 * nh:512 * (nh + 1)],
                        in_=xb_d[t * 128:(t + 1) * 128, 512 * nh:512 * (nh + 1)])
                if nh == 1:
                    # wv is first needed by the V projection of quarter 0;
                    # keeping it behind the first two xb 16ths-pairs gets the
                    # Q/K inputs in ~1us earlier
                    nc.scalar.dma_start(
                        out=wv_sb, in_=wvt_d[:].rearrange("(t p) m -> p t m", p=128))
            if has_bq:
                bq_sb = const.tile([128, 1], F32)
                nc.sync.dma_start(out=bq_sb, in_=bq_d[:, :])
            if has_bk:
                bk_sb = const.tile([128, 1], F32)
                nc.sync.dma_start(out=bk_sb, in_=bk_d[:, :])
            if has_bv:
                bv_sb = const.tile([128, C], F32)
                nc.sync.dma_start(out=bv_sb, in_=bv_d[:, :])
            q_rep = acts.tile([128, MQ], BF16)
            k_rep = acts.tile([128, N], BF16)
            vhat = acts.tile([128, NBLK, C + 1], BF16)

            # psum_e lives for the whole kernel (strip-0 energy overlaps the
            # projections); psum_p is released before psum_o is allocated.
            psum_e = tc.alloc_tile_pool(name="psum_e", bufs=2, space="PSUM")
            pts = {}     # (st, g) -> [pt_half0, pt_half1]
            pso_by_strip = {}
            e_pools = [psum_e, psum_e]
            e_cnt = [0]

            def emit_e(st, g):
                """Energy^T + exp for key blocks 4g..4g+3 of strip st."""
                qsl = slice(MQ_CHUNK * st, MQ_CHUNK * (st + 1))
                row = []
                for half in range(2):
                    pool = e_pools[e_cnt[0] % len(e_pools)]
                    e_cnt[0] += 1
                    pse = pool.tile([128, 1024], F32, tag="pse", name="pse")
                    for jj in range(2):
                        j = 2 * half + jj
                        blk = 4 * g + j
                        nc.tensor.matmul(
                            pse[:, 512 * jj:512 * (jj + 1)],
                            lhsT=k_rep[32 * j:32 * (j + 1), 128 * blk:128 * (blk + 1)],
                            rhs=q_rep[32 * j:32 * (j + 1), qsl],
                            start=True, stop=True,
                            tile_position=(32 * j, 0),
                        )
                    pt = ptp.tile([128, 1024], BF16, tag="pt", name="pt")
                    nc.scalar.activation(pt, pse, func=mybir.ActivationFunctionType.Exp)
                    row.append(pt)
                pts[(st, g)] = row

            with tc.tile_pool(name="psum_p", bufs=2, space="PSUM") as psum_p:
                # PE warm-up while input DMAs are in flight: HAM releases the
                # PE clock gate (1.2 -> 2.4 GHz) after ~3.4us of sustained
                # matmul activity, so burn the DMA wait on dummy matmuls and
                # the real projections start warm.
                warm = const.tile([128, 512], BF16)
                nc.vector.memset(warm, 0.0)
                # Pull the exp ACT_TABLE_LOAD (~2.7us) off the critical path:
                # a dummy exp during the DMA wait loads the table set early.
                warm_exp = const.tile([128, 1], F32)
                nc.scalar.activation(warm_exp, warm[:, 0:1],
                                     func=mybir.ActivationFunctionType.Exp)
                def fill(n):
                    # keep the PE busy (HAM warm) across DMA-wait windows
                    for _ in range(n):
                        psw = psum_p.tile([128, 512], F32, tag="pp", name="psw")
                        nc.tensor.matmul(psw, lhsT=warm[:, 0:128], rhs=warm,
                                         start=True, stop=True)

                fill(WARMUP_MMS)
                # Q projection: q_rep[32g+d, i] = q[d, i] (4 copies)
                for mc in range(MQ // 512):
                    psq = psum_p.tile([128, 512], F32, tag="pp")
                    sl = slice(512 * mc, 512 * (mc + 1))
                    for cc in range(2):
                        nc.tensor.matmul(psq, lhsT=wq_sb[:, cc, :], rhs=xb_sb[:, cc, sl],
                                         start=(cc == 0), stop=(cc == 1))
                    if has_bq:
                        nc.vector.tensor_scalar_add(q_rep[:, sl], psq, bq_sb)
                    else:
                        nc.vector.tensor_copy(q_rep[:, sl], psq)
                pass  # K reads the same xb 16ths as Q - no fill needed
                # K and V^T projections in xb-quarter arrival order, with
                # strip-0 energy groups interleaved (ScalarE runs their exps
                # while the PE chews projections). All projection copies go to
                # DVE so ScalarE is exp-only.
                # V^T: vhat[j, c] = gamma * v[c, j]; col 256 = 1.0. Two V
                # blocks share one PSUM bank (per-element has_written bits
                # make co-resident accumulation groups safe), so each pair
                # needs one PSUM->SBUF copy.
                nc.vector.memset(vhat[:, :, C:C + 1], 1.0)
                for qt in range(4):
                    for mc in range(2 * qt, 2 * qt + 2):
                        psk = psum_p.tile([128, 512], F32, tag="pp")
                        sl = slice(512 * mc, 512 * (mc + 1))
                        for cc in range(2):
                            nc.tensor.matmul(psk, lhsT=wk_sb[:, cc, :],
                                             rhs=xb_sb[:, cc, sl],
                                             start=(cc == 0), stop=(cc == 1))
                        if has_bk:
                            nc.vector.tensor_scalar_add(k_rep[:, sl], psk, bk_sb)
                        else:
                            nc.vector.tensor_copy(k_rep[:, sl], psk)
                        emit_e(0, mc)
                    for np_ in range(4 * qt, 4 * qt + 4):
                        psv = psum_p.tile([128, 512], F32, tag="pv")
                        for half in range(2):
                            nb = 2 * np_ + half
                            for cc in range(2):
                                nc.tensor.matmul(
                                    psv[:, 256 * half:256 * (half + 1)],
                                    lhsT=xb_sb[:, cc, 128 * nb:128 * (nb + 1)],
                                    rhs=wv_sb[:, cc, :],
                                    start=(cc == 0), stop=(cc == 1),
                                    skip_group_check=True)
                        if has_bv:
                            for half in range(2):
                                nc.vector.tensor_add(
                                    vhat[:, 2 * np_ + half, 0:C],
                                    psv[:, 256 * half:256 * (half + 1)], bv_sb)
                        else:
                            nc.vector.tensor_copy(vhat[:, 2 * np_:2 * np_ + 2, 0:C],
                                                  psv.rearrange("p (b c) -> p b c", b=2))

            # Last strip's residual comes via SBUF (mid-kernel prefetch, idle
            # DMA window) so the kernel tail isn't gated by slow SWDGE
            # accumulate-DMAs; earlier strips use accumulate-DMA reads.
            xqt3_sb = acts.tile([128, 4, C], F32)
            nc.sync.dma_start(
                out=xqt3_sb,
                in_=xqt_d[MQ - 512:MQ, :].rearrange("(s p) m -> p s m", p=128))

            # ---- attention strips (one flat cross-strip pipeline) ----------
            psum_o = tc.alloc_tile_pool(name="psum_o", bufs=1, space="PSUM")

            def emit_av(st, g):
                """Accumulate P@[V^T|1] for key blocks 4g..4g+3 of strip st."""
                if g == 0:
                    pso_by_strip[st] = [
                        psum_o.tile([128, C + 1], F32, tag=f"o{s}", name=f"pso{s}")
                        for s in range(4)]
                pso = pso_by_strip[st]
                for s in range(4):
                    for j in range(4):
                        blk = 4 * g + j
                        pt = pts[(st, g)][j // 2]
                        col = 512 * (j % 2) + 128 * s
                        nc.tensor.matmul(
                            pso[s], lhsT=pt[:, col:col + 128],
                            rhs=vhat[:, blk, :],
                            start=(blk == 0), stop=(blk == NBLK - 1),
                        )
                del pts[(st, g)]

            def emit_out(st):
                """Normalize + residual + store strip st."""
                pso = pso_by_strip.pop(st)
                last = st == NSTRIP - 1
                osb = outp.tile([128, 4, C], F32, tag="osb", name="osb")
                for s in range(4):
                    qb = 4 * st + s
                    rec = outp.tile([128, 1], F32, tag="rec", name="rec")
                    nc.vector.reciprocal(rec, pso[s][:, C:C + 1])
                    if last:
                        # normalize on ACT (idle at the tail), residual from
                        # the prefetched SBUF copy, store per-subblock
                        nc.scalar.activation(osb[:, s, :], pso[s][:, 0:C],
                                             func=mybir.ActivationFunctionType.Copy,
                                             scale=rec)
                        nc.vector.tensor_add(osb[:, s, :], osb[:, s, :],
                                             xqt3_sb[:, s, :])
                        nc.sync.dma_start(out=out_d[128 * qb:128 * (qb + 1), :],
                                          in_=osb[:, s, :])
                    else:
                        nc.vector.tensor_scalar_mul(osb[:, s, :], pso[s][:, 0:C], rec)
                        # residual add via accumulating DMA read of x^T: keeps
                        # the 2MB fp32 residual off the startup DMA window
                        nc.gpsimd.dma_start(out=osb[:, s, :],
                                            in_=xqt_d[128 * qb:128 * (qb + 1), :],
                                            accum_op=mybir.AluOpType.add)
                if not last:
                    nc.sync.dma_start(
                        out=out_d[512 * st:512 * (st + 1), :].rearrange(
                            "(s p) m -> p s m", p=128),
                        in_=osb)

            # strip 0's energy groups were emitted during the projections; the
            # next strip's energy trickles uniformly one-group-behind the
            # current strip's AV, so ScalarE's exp chain never starves (the
            # PSUM slot for E(st+1, g) frees exactly when exp(st+1, g-1)
            # completes, just ahead of the PE reaching it).
            for st in range(NSTRIP):
                for g in range(8):
                    emit_av(st, g)
                    if st + 1 < NSTRIP:
                        emit_e(st + 1, g)
                emit_out(st)
            psum_o.release()
            psum_e.release()

    if not nc.is_finalized():
        nc.finalize()
    return nc


def kernel(x, Wq, bq, Wk, bk, Wv, bv, gamma):
    global last_results
    x = np.asarray(x, dtype=np.float32)
    Wq = np.asarray(Wq, dtype=np.float32)
    Wk = np.asarray(Wk, dtype=np.float32)
    Wv = np.asarray(Wv, dtype=np.float32)
    bq = np.asarray(bq, dtype=np.float32)
    bk = np.asarray(bk, dtype=np.float32)
    bv = np.asarray(bv, dtype=np.float32)
    gamma_v = float(np.asarray(gamma).reshape(-1)[0])
    assert x.shape == (B, C, H, W)

    scale = 1.0 / np.sqrt(C)
    has_bq = bool(np.any(bq != 0))
    has_bk = bool(np.any(bk != 0))
    has_bv = bool(np.any(bv != 0))

    key = (has_bq, has_bk, has_bv)
    if key not in _nc_cache:
        _nc_cache[key] = _build_nc(*key)
    nc = _nc_cache[key]

    bf = ml_dtypes.bfloat16
    wqt4 = np.tile(Wq.T * scale, (1, 4)).astype(bf)          # [C, 128]
    wkt4 = np.tile(Wk.T, (1, 4)).astype(bf)                  # [C, 128]
    wvt = (Wv.T * gamma_v).astype(bf)                        # [C, C]

    xf = x.reshape(B, C, N)
    in_maps = []
    for core in range(NCORES):
        b, half = divmod(core, 2)
        qsl = slice(half * MQ, (half + 1) * MQ)
        # rotate the core's query columns to the front; softmax over keys is
        # permutation-invariant so key order doesn't matter
        xrot = np.roll(xf[b], -half * MQ, axis=1) if half else xf[b]
        m = {
            "xb": xrot.astype(bf),
            "xqt": np.ascontiguousarray(xf[b][:, qsl].T),
            "wqt4": wqt4,
            "wkt4": wkt4,
            "wvt": wvt,
        }
        if has_bq:
            m["bq4"] = np.tile(bq * scale, 4).reshape(128, 1).astype(np.float32)
        if has_bk:
            m["bk4"] = np.tile(bk, 4).reshape(128, 1).astype(np.float32)
        if has_bv:
            m["bvg"] = np.broadcast_to(bv * gamma_v, (128, C)).astype(np.float32).copy()
        in_maps.append(m)

    trace = bool(os.environ.get("BASS_TRACE"))
    if trace:
        try:
            import antenv.axon_hooks  # noqa: F401
        except ImportError:
            trace = False
    tmpdir = os.environ.get("BASS_KERNEL_TMPDIR") or None
    res = run_bass_kernel_spmd(nc, in_maps, list(range(NCORES)), trace=trace,
                               tmpdir=tmpdir)
    last_results = res

    out = np.empty((B, C, N), dtype=np.float32)
    for core in range(NCORES):
        b, half = divmod(core, 2)
        out[b, :, half * MQ:(half + 1) * MQ] = res.results[core]["out"].T
    return out.reshape(B, C, H, W)



# revision 5
# speedup vs baseline: 1.1713x; 1.1713x over previous
"""Fused single-head CNN self-attention kernel for Trainium2 (8 NeuronCores).

Computes, per batch b:
    q = (Wq/sqrt(C)) @ x + bq/sqrt(C)   (Cqk=32, N=4096)
    k = Wk @ x + bk
    v = Wv @ x + bv
    E[i, j]  = q[:, i] . k[:, j]        (already scaled by 1/sqrt(C))
    P        = softmax_j(E)
    out[c,i] = gamma * sum_j P[i, j] v[c, j] + x[c, i]

Sharding: B=4 batches x 2 query-halves -> 8 cores, no cross-core comms.
Each core handles one batch's full keys/values and 2048 queries.

Device-side layout tricks:
  * Wq^T/Wk^T are replicated 4x along columns on the host so the Q/K
    projections produce Q_rep/K_rep with 4 partition-group copies; the
    energy matmul (contraction depth 32) then packs 4 concurrent
    matmuls into the 128x128 PE array via tile_position row tiling.
  * Energy is computed transposed, E^T[key, query], so exp(E^T) is
    already the stationary operand layout the P@V matmul needs; no
    transposes anywhere in the kernel.
  * V is projected directly transposed (V^T[n, c]) with an extra ones
    column, so the P@V matmul's PSUM output column 256 accumulates the
    softmax denominator for free.
  * Softmax skips max-subtraction: E = q.k/sqrt(C) with unit-variance
    inputs is bounded (|E| < ~3), far from fp32 exp overflow.
"""

import os

import numpy as np
import ml_dtypes

import concourse.bass as bass
import concourse.mybir as mybir
from concourse import bacc
from concourse.tile import TileContext
from concourse.bass_utils import run_bass_kernel_spmd

# Problem shape (hardcoded per contest contract).
B, C, H, W = 4, 256, 64, 64
N = H * W          # 4096 keys per batch
D = 32             # q/k head dim
NCORES = 8
MQ = N // 2        # 2048 queries per core
MQ_CHUNK = 512     # query strip width (PSUM bank = 512 fp32)
NBLK = N // 128    # 32 key blocks
NSTRIP = MQ // MQ_CHUNK  # 4

F32 = mybir.dt.float32
BF16 = mybir.dt.bfloat16
F8 = mybir.dt.float8e4
DR = mybir.MatmulPerfMode.DoubleRow
VPAD = 272  # vhat free-dim stride: C+1 rounded up to 16B (DoubleRow AP align)
WARMUP_MMS = int(os.environ.get("KERNEL_WARMUP_MMS", "7"))

# Module-level stash of the last run's results (exec_time_ns etc.) so a
# test harness can report HW time without changing kernel()'s signature.
last_results = None
_nc_cache = {}


def _build_nc(has_bq, has_bk, has_bv):
    nc = bacc.Bacc(None)

    # xb is the core's batch with its 2048 query columns rotated to the
    # front (softmax over keys is permutation-invariant), so the query
    # slice is the compile-time-constant columns 0:MQ of xb.
    xb_d = nc.declare_dram_parameter("xb", [C, N], BF16, isOutput=False)
    xqt_d = nc.declare_dram_parameter("xqt", [MQ, C], F32, isOutput=False)
    wqt_d = nc.declare_dram_parameter("wqt4", [C, 128], BF16, isOutput=False)
    wkt_d = nc.declare_dram_parameter("wkt4", [C, 128], BF16, isOutput=False)
    wvt_d = nc.declare_dram_parameter("wvt", [C, C], BF16, isOutput=False)
    if has_bq:
        bq_d = nc.declare_dram_parameter("bq4", [128, 1], F32, isOutput=False)
    if has_bk:
        bk_d = nc.declare_dram_parameter("bk4", [128, 1], F32, isOutput=False)
    if has_bv:
        bv_d = nc.declare_dram_parameter("bvg", [128, C], F32, isOutput=False)
    out_d = nc.declare_dram_parameter("out", [MQ, C], F32, isOutput=True)

    with TileContext(nc) as tc:
        with (
            tc.tile_pool(name="const", bufs=1) as const,
            tc.tile_pool(name="acts", bufs=1) as acts,
            tc.tile_pool(name="ptp", bufs=26) as ptp,
            tc.tile_pool(name="outp", bufs=4) as outp,
        ):
            # ---- load weights + activations --------------------------------
            # One DMA trigger per tensor (triggers cost ~600ns each on SyncE
            # and serialize); order so Q-projection inputs land first.
            wq_sb = const.tile([128, 2, 128], BF16)
            wk_sb = const.tile([128, 2, 128], BF16)
            wv_sb = const.tile([128, 2, C], BF16)
            xb_sb = acts.tile([128, 2, N], BF16)
            # Spread loads across the two HWDGE queues (sync + scalar);
            # xb eighths land in consumption order (queries = quarters 0-1).
            nc.sync.dma_start(out=wq_sb, in_=wqt_d[:].rearrange("(t p) m -> p t m", p=128))
            nc.scalar.dma_start(out=wk_sb, in_=wkt_d[:].rearrange("(t p) m -> p t m", p=128))
            for nh in range(8):
                for t in range(2):
                    eng = nc.sync if t == 0 else nc.scalar
                    eng.dma_start(
                        out=xb_sb[:, t, 512 * nh:512 * (nh + 1)],
                        in_=xb_d[t * 128:(t + 1) * 128, 512 * nh:512 * (nh + 1)])
                if nh == 1:
                    # wv is first needed by the V projection of quarter 0;
                    # keeping it behind the first two xb 16ths-pairs gets the
                    # Q/K inputs in ~1us earlier
                    nc.scalar.dma_start(
                        out=wv_sb, in_=wvt_d[:].rearrange("(t p) m -> p t m", p=128))
            if has_bq:
                bq_sb = const.tile([128, 1], F32)
                nc.sync.dma_start(out=bq_sb, in_=bq_d[:, :])
            if has_bk:
                bk_sb = const.tile([128, 1], F32)
                nc.sync.dma_start(out=bk_sb, in_=bk_d[:, :])
            if has_bv:
                bv_sb = const.tile([128, C], F32)
                nc.sync.dma_start(out=bv_sb, in_=bv_d[:, :])
            q_rep = acts.tile([128, MQ], BF16)
            k_rep = acts.tile([128, N], BF16)
            vhat = acts.tile([128, NBLK, VPAD], F8)

            # psum_e lives for the whole kernel (strip-0 energy overlaps the
            # projections); psum_p is released before psum_o is allocated.
            psum_e = tc.alloc_tile_pool(name="psum_e", bufs=2, space="PSUM")
            pts = {}     # (st, g) -> [pt_half0, pt_half1]
            pso_by_strip = {}
            e_pools = [psum_e, psum_e]
            e_cnt = [0]

            def emit_e(st, g):
                """Energy^T + exp for key blocks 4g..4g+3 of strip st."""
                qsl = slice(MQ_CHUNK * st, MQ_CHUNK * (st + 1))
                row = []
                for half in range(2):
                    pool = e_pools[e_cnt[0] % len(e_pools)]
                    e_cnt[0] += 1
                    pse = pool.tile([128, 2, 512], F32, tag="pse", name="pse")
                    for jj in range(2):
                        j = 2 * half + jj
                        blk = 4 * g + j
                        nc.tensor.matmul(
                            pse[:, jj, :],
                            lhsT=k_rep[32 * j:32 * (j + 1), 128 * blk:128 * (blk + 1)],
                            rhs=q_rep[32 * j:32 * (j + 1), qsl],
                            start=True, stop=True,
                            tile_position=(32 * j, 0),
                        )
                    pt = ptp.tile([128, 2, 512], F8, tag="pt", name="pt")
                    nc.scalar.activation(pt, pse, func=mybir.ActivationFunctionType.Exp)
                    row.append(pt)
                pts[(st, g)] = row

            with tc.tile_pool(name="psum_p", bufs=2, space="PSUM") as psum_p:
                # PE warm-up while input DMAs are in flight: HAM releases the
                # PE clock gate (1.2 -> 2.4 GHz) after ~3.4us of sustained
                # matmul activity, so burn the DMA wait on dummy matmuls and
                # the real projections start warm.
                warm = const.tile([128, 512], BF16)
                nc.vector.memset(warm, 0.0)
                # Pull the exp ACT_TABLE_LOAD (~2.7us) off the critical path:
                # a dummy exp during the DMA wait loads the table set early.
                warm_exp = const.tile([128, 1], F32)
                nc.scalar.activation(warm_exp, warm[:, 0:1],
                                     func=mybir.ActivationFunctionType.Exp)
                def fill(n):
                    # keep the PE busy (HAM warm) across DMA-wait windows
                    for _ in range(n):
                        psw = psum_p.tile([128, 512], F32, tag="pp", name="psw")
                        nc.tensor.matmul(psw, lhsT=warm[:, 0:128], rhs=warm,
                                         start=True, stop=True)

                fill(WARMUP_MMS)
                # Q projection: q_rep[32g+d, i] = q[d, i] (4 copies)
                for mc in range(MQ // 512):
                    psq = psum_p.tile([128, 512], F32, tag="pp")
                    sl = slice(512 * mc, 512 * (mc + 1))
                    for cc in range(2):
                        nc.tensor.matmul(psq, lhsT=wq_sb[:, cc, :], rhs=xb_sb[:, cc, sl],
                                         start=(cc == 0), stop=(cc == 1))
                    if has_bq:
                        nc.vector.tensor_scalar_add(q_rep[:, sl], psq, bq_sb)
                    else:
                        nc.vector.tensor_copy(q_rep[:, sl], psq)
                pass  # K reads the same xb 16ths as Q - no fill needed
                # K and V^T projections in xb-quarter arrival order, with
                # strip-0 energy groups interleaved (ScalarE runs their exps
                # while the PE chews projections). All projection copies go to
                # DVE so ScalarE is exp-only.
                # V^T: vhat[j, c] = gamma * v[c, j]; col 256 = 1.0. Two V
                # blocks share one PSUM bank (per-element has_written bits
                # make co-resident accumulation groups safe), so each pair
                # needs one PSUM->SBUF copy.
                nc.vector.memset(vhat[:, :, C:C + 1], 1.0)
                for qt in range(4):
                    for mc in range(2 * qt, 2 * qt + 2):
                        psk = psum_p.tile([128, 512], F32, tag="pp")
                        sl = slice(512 * mc, 512 * (mc + 1))
                        for cc in range(2):
                            nc.tensor.matmul(psk, lhsT=wk_sb[:, cc, :],
                                             rhs=xb_sb[:, cc, sl],
                                             start=(cc == 0), stop=(cc == 1))
                        if has_bk:
                            nc.vector.tensor_scalar_add(k_rep[:, sl], psk, bk_sb)
                        else:
                            nc.vector.tensor_copy(k_rep[:, sl], psk)
                        emit_e(0, mc)
                    for np_ in range(4 * qt, 4 * qt + 4):
                        psv = psum_p.tile([128, 512], F32, tag="pv")
                        for half in range(2):
                            nb = 2 * np_ + half
                            for cc in range(2):
                                nc.tensor.matmul(
                                    psv[:, 256 * half:256 * (half + 1)],
                                    lhsT=xb_sb[:, cc, 128 * nb:128 * (nb + 1)],
                                    rhs=wv_sb[:, cc, :],
                                    start=(cc == 0), stop=(cc == 1),
                                    skip_group_check=True)
                        if has_bv:
                            for half in range(2):
                                nc.vector.tensor_add(
                                    vhat[:, 2 * np_ + half, 0:C],
                                    psv[:, 256 * half:256 * (half + 1)], bv_sb)
                        else:
                            nc.vector.tensor_copy(vhat[:, 2 * np_:2 * np_ + 2, 0:C],
                                                  psv.rearrange("p (b c) -> p b c", b=2))

            # Last strip's residual comes via SBUF (mid-kernel prefetch, idle
            # DMA window) so the kernel tail isn't gated by slow SWDGE
            # accumulate-DMAs; earlier strips use accumulate-DMA reads.
            xqt3_sb = acts.tile([128, 4, C], F32)
            nc.sync.dma_start(
                out=xqt3_sb,
                in_=xqt_d[MQ - 512:MQ, :].rearrange("(s p) m -> p s m", p=128))

            # ---- attention strips (one flat cross-strip pipeline) ----------
            psum_o = tc.alloc_tile_pool(name="psum_o", bufs=1, space="PSUM")

            def emit_av(st, g):
                """Accumulate P@[V^T|1] for key blocks 4g..4g+3 of strip st."""
                if g == 0:
                    pso_by_strip[st] = [
                        psum_o.tile([128, C + 1], F32, tag=f"o{s}", name=f"pso{s}")
                        for s in range(4)]
                pso = pso_by_strip[st]
                for s in range(4):
                    for h in range(2):
                        blk0 = 4 * g + 2 * h
                        pt = pts[(st, g)][h]
                        nc.tensor.matmul(
                            pso[s], lhsT=pt[:, :, 128 * s:128 * (s + 1)],
                            rhs=vhat[:, blk0:blk0 + 2, 0:C + 1],
                            start=(blk0 == 0), stop=(blk0 == NBLK - 2),
                            perf_mode=DR,
                        )
                del pts[(st, g)]

            def emit_out(st):
                """Normalize + residual + store strip st."""
                pso = pso_by_strip.pop(st)
                last = st == NSTRIP - 1
                osb = outp.tile([128, 4, C], F32, tag="osb", name="osb")
                for s in range(4):
                    qb = 4 * st + s
                    rec = outp.tile([128, 1], F32, tag="rec", name="rec")
                    nc.vector.reciprocal(rec, pso[s][:, C:C + 1])
                    if last:
                        # normalize on ACT (idle at the tail), residual from
                        # the prefetched SBUF copy, store per-subblock
                        nc.scalar.activation(osb[:, s, :], pso[s][:, 0:C],
                                             func=mybir.ActivationFunctionType.Copy,
                                             scale=rec)
                        nc.vector.tensor_add(osb[:, s, :], osb[:, s, :],
                                             xqt3_sb[:, s, :])
                        nc.sync.dma_start(out=out_d[128 * qb:128 * (qb + 1), :],
                                          in_=osb[:, s, :])
                    else:
                        nc.vector.tensor_scalar_mul(osb[:, s, :], pso[s][:, 0:C], rec)
                        # residual add via accumulating DMA read of x^T: keeps
                        # the 2MB fp32 residual off the startup DMA window
                        nc.gpsimd.dma_start(out=osb[:, s, :],
                                            in_=xqt_d[128 * qb:128 * (qb + 1), :],
                                            accum_op=mybir.AluOpType.add)
                if not last:
                    nc.sync.dma_start(
                        out=out_d[512 * st:512 * (st + 1), :].rearrange(
                            "(s p) m -> p s m", p=128),
                        in_=osb)

            # strip 0's energy groups were emitted during the projections; the
            # next strip's energy trickles uniformly one-group-behind the
            # current strip's AV, so ScalarE's exp chain never starves (the
            # PSUM slot for E(st+1, g) frees exactly when exp(st+1, g-1)
            # completes, just ahead of the PE reaching it).
            for st in range(NSTRIP):
                for g in range(8):
                    emit_av(st, g)
                    if st + 1 < NSTRIP:
                        emit_e(st + 1, g)
                emit_out(st)
            psum_o.release()
            psum_e.release()

    if not nc.is_finalized():
        nc.finalize()
    return nc


def kernel(x, Wq, bq, Wk, bk, Wv, bv, gamma):
    global last_results
    x = np.asarray(x, dtype=np.float32)
    Wq = np.asarray(Wq, dtype=np.float32)
    Wk = np.asarray(Wk, dtype=np.float32)
    Wv = np.asarray(Wv, dtype=np.float32)
    bq = np.asarray(bq, dtype=np.float32)
    bk = np.asarray(bk, dtype=np.float32)
    bv = np.asarray(bv, dtype=np.float32)
    gamma_v = float(np.asarray(gamma).reshape(-1)[0])
    assert x.shape == (B, C, H, W)

    scale = 1.0 / np.sqrt(C)
    has_bq = bool(np.any(bq != 0))
    has_bk = bool(np.any(bk != 0))
    has_bv = bool(np.any(bv != 0))

    key = (has_bq, has_bk, has_bv)
    if key not in _nc_cache:
        _nc_cache[key] = _build_nc(*key)
    nc = _nc_cache[key]

    bf = ml_dtypes.bfloat16
    wqt4 = np.tile(Wq.T * scale, (1, 4)).astype(bf)          # [C, 128]
    wkt4 = np.tile(Wk.T, (1, 4)).astype(bf)                  # [C, 128]
    wvt = (Wv.T * gamma_v).astype(bf)                        # [C, C]

    xf = x.reshape(B, C, N)
    in_maps = []
    for core in range(NCORES):
        b, half = divmod(core, 2)
        qsl = slice(half * MQ, (half + 1) * MQ)
        # rotate the core's query columns to the front; softmax over keys is
        # permutation-invariant so key order doesn't matter
        xrot = np.roll(xf[b], -half * MQ, axis=1) if half else xf[b]
        m = {
            "xb": xrot.astype(bf),
            "xqt": np.ascontiguousarray(xf[b][:, qsl].T),
            "wqt4": wqt4,
            "wkt4": wkt4,
            "wvt": wvt,
        }
        if has_bq:
            m["bq4"] = np.tile(bq * scale, 4).reshape(128, 1).astype(np.float32)
        if has_bk:
            m["bk4"] = np.tile(bk, 4).reshape(128, 1).astype(np.float32)
        if has_bv:
            m["bvg"] = np.broadcast_to(bv * gamma_v, (128, C)).astype(np.float32).copy()
        in_maps.append(m)

    trace = bool(os.environ.get("BASS_TRACE"))
    if trace:
        try:
            import antenv.axon_hooks  # noqa: F401
        except ImportError:
            trace = False
    tmpdir = os.environ.get("BASS_KERNEL_TMPDIR") or None
    res = run_bass_kernel_spmd(nc, in_maps, list(range(NCORES)), trace=trace,
                               tmpdir=tmpdir)
    last_results = res

    out = np.empty((B, C, N), dtype=np.float32)
    for core in range(NCORES):
        b, half = divmod(core, 2)
        out[b, :, half * MQ:(half + 1) * MQ] = res.results[core]["out"].T
    return out.reshape(B, C, H, W)

